# revision 27
# baseline (speedup 1.0000x reference)
"""Trainium2 Bass kernel for a 2-layer GCN encoder (GCNConv -> ReLU -> {GCNConv mu, GCNConv logstd}).

Strategy (8 NeuronCores, SPMD):
  - Math: propagate(M) = D^-1/2 (A+I) D^-1/2 M  ==  d * ((A+I) @ (d * M)) with d = deg^-1/2,
    so per-edge norm weights disappear: scale rows by d before and after message passing.
  - Layers 2 and 3 share the propagate: fuse W_mu/W_logstd into one [128,128] matmul + one
    message-passing pass over 128 features, split on the host afterwards.
  - Sharding: nodes are partitioned across the 8 cores (dst-sharding). Each core owns
    N/8 = 6250 output rows and processes the ~E/8 edges pointing into them.
  - Layer-1 linear (x @ W1.T) is sharded: each core computes NPAD/8 table rows, then an
    AllGather (direct to plain DRAM) replicates the table for gathering. Layer-2 likewise.
  - Message passing: dma_gather (HW gather, int16 indices) pulls PAIR rows (2 nodes, 512B)
    from the HBM table viewed as [NPAD/2, 256]; idx = src>>1 < 32767 so no table split.
    Edges are bucketed by (dst tile, src parity) so each 128-edge chunk uses one parity
    half of the gathered rows as matmul lhsT. A one-hot selection matrix (DVE is_equal vs
    iota) turns the segment-sum into PE matmuls accumulated in PSUM per 128-dst-node tile.
  - Gathers are issued round-robin on 4 SWDGE queues: descriptor generation for different
    queues runs concurrently on different Q7 core groups (~3x faster than one queue).
  - All cores run the same program (SPMD): per-(tile,parity) group sizes padded to the
    max over cores.

kernel(**inputs) takes the full-size inputs and returns (mu, logstd) as float32 numpy arrays.
"""
import sys

sys.path.insert(0, "/opt/trn_rl_repo")

import numpy as np
import ml_dtypes

import concourse.bass as bass
import concourse.bacc as bacc
import concourse.mybir as mybir
import concourse.tile as tile
from concourse.bass_utils import run_bass_kernel_spmd

BF16 = ml_dtypes.bfloat16

# ---------------- configuration ----------------
FULL_CFG = dict(
    n=50000,        # nodes
    fin=512,        # input features
    hid=128,        # hidden features
    out2=128,       # fused mu+logstd features
    n_cores=8,
    npad=53248,     # padded node count (multiple of 512*8; NSH=6656=13*512 per core)
    blk=512,        # phase-A block width
    g_edges=4096,   # steady-state gather size (edges per dma_gather)
    g_ramp=(1024, 1024, 1024, 1024, 2048, 2048),  # startup ramp sizes
    swdge_queues=4,
    gather_bufs=5,  # bufs for the steady-state gather size
    queue_rr=(0, 1, 2, 3),  # round-robin queue assignment for gathers
    ag_shared=False,        # AllGather to Shared scratchpad + bounce (fallback)
)


def _ceil(a, b):
    return -(-a // b)


def preprocess(cfg, x, edge_index, W1, b1, W_mu, b_mu, W_logstd, b_logstd):
    """Host-side: degrees, edge bucketing/padding, operand staging. Returns
    (meta, in_maps). Pure index/layout work plus parameter reformatting."""
    N, C = cfg["n"], cfg["n_cores"]
    NPC = N // C
    T = _ceil(NPC, 128)
    NPAD = cfg["npad"]
    NSH = NPAD // C

    x = np.asarray(x, np.float32)
    ei = np.asarray(edge_index).astype(np.int64)
    W1 = np.asarray(W1, np.float32)
    b1 = np.asarray(b1, np.float32)
    Wcat = np.concatenate([np.asarray(W_mu, np.float32), np.asarray(W_logstd, np.float32)], axis=0)
    bcat = np.concatenate([np.asarray(b_mu, np.float32), np.asarray(b_logstd, np.float32)], axis=0)

    src = np.concatenate([ei[0], np.arange(N, dtype=np.int64)])
    dst = np.concatenate([ei[1], np.arange(N, dtype=np.int64)])
    deg = np.bincount(dst, minlength=N).astype(np.float32)
    dvec = (1.0 / np.sqrt(deg)).astype(np.float32)

    core = dst // NPC
    tloc = (dst % NPC) // 128
    par = (src & 1).astype(np.int64)
    key = (core * T + tloc) * 2 + par
    order = np.argsort(key, kind="stable")
    ss, ds = src[order], dst[order]
    counts = np.bincount(key, minlength=C * T * 2).reshape(C, T, 2)
    gpad = ((counts.max(axis=0) + 127) // 128) * 128  # [T, 2] padded group sizes
    L = int(gpad.sum())
    K_tot = L // 128
    offs = np.concatenate([[0], np.cumsum(counts.reshape(-1))])

    # phase-A staging (per-core x slice)
    xt_full = np.zeros((cfg["fin"], NPAD), BF16)
    xt_full[:, :N] = x.T
    w1t = np.ascontiguousarray(W1.T).astype(BF16)          # [fin, hid]
    wcatt = np.ascontiguousarray(Wcat.T).astype(BF16)      # [hid, out2]
    d_all = np.ones(NPAD, np.float32)
    d_all[:N] = dvec
    iota_arr = np.tile(np.arange(128), (128, 1)).astype(BF16)
    ident = np.eye(128, dtype=BF16)

    in_maps = []
    for c in range(C):
        idxbuf = np.zeros(L, np.int16)
        dstloc_all = np.full(L, 200, np.int32)
        pos = 0
        for t in range(T):
            for h in (0, 1):
                g = int(counts[c, t, h])
                o = int(offs[(c * T + t) * 2 + h])
                sl = slice(o, o + g)
                idxbuf[pos:pos + g] = (ss[sl] >> 1).astype(np.int16)
                dstloc_all[pos:pos + g] = (ds[sl] % NPC) - t * 128
                pos += int(gpad[t, h])
        idx_w = np.tile(idxbuf.reshape(-1, 16).T, (8, 1)).copy()
        dstloc_arr = np.ascontiguousarray(dstloc_all.reshape(-1, 128).T).astype(BF16)

        d_own = dvec[c * NPC:(c + 1) * NPC]
        d_own_pad = np.ones(T * 128, np.float32)
        d_own_pad[:NPC] = d_own
        d_rep = np.tile(d_own_pad, (128, 1)).astype(np.float32)          # [128, T*128]
        d_own_col = np.ascontiguousarray(d_own_pad.reshape(-1, 128).T)   # [128, T]

        d_sh = d_all[c * NSH:(c + 1) * NSH]
        d_sh_col = np.ascontiguousarray(d_sh.reshape(-1, 128).T)         # [128, NSH/128]

        in_maps.append({
            "xt": np.ascontiguousarray(xt_full[:, c * NSH:(c + 1) * NSH]),
            "w1t": w1t, "wcatt": wcatt,
            "b1c": b1.reshape(-1, 1).copy(), "bcatc": bcat.reshape(-1, 1).copy(),
            "dshc": d_sh_col, "drep": d_rep, "downc": d_own_col,
            "iota": iota_arr, "ident": ident,
            "idx": idx_w, "dstloc": dstloc_arr,
        })

    gpt = [[int(gpad[t, 0]) // 128, int(gpad[t, 1]) // 128] for t in range(T)]
    meta = dict(gpt=gpt, L=L, K_tot=K_tot)
    return meta, in_maps


def build_program(cfg, meta):
    N, C = cfg["n"], cfg["n_cores"]
    NPC = N // C
    T = _ceil(NPC, 128)
    NPAD = cfg["npad"]
    NSH = NPAD // C
    FIN, HID, O2 = cfg["fin"], cfg["hid"], cfg["out2"]
    BLK, G = cfg["blk"], cfg["g_edges"]
    KC = FIN // 128
    SPC = G // 128
    gpt, L, K_tot = meta["gpt"], meta["L"], meta["K_tot"]
    bf16 = mybir.dt.bfloat16
    f32 = mybir.dt.float32
    AF = mybir.ActivationFunctionType
    OP = mybir.AluOpType

    nc = bacc.Bacc("TRN2", target_bir_lowering=False, debug=False, num_devices=C,
                   num_swdge_queues=cfg["swdge_queues"])

    xt_d = nc.dram_tensor("xt", [FIN, NSH], bf16, kind="ExternalInput")
    w1t_d = nc.dram_tensor("w1t", [FIN, HID], bf16, kind="ExternalInput")
    wcatt_d = nc.dram_tensor("wcatt", [HID, O2], bf16, kind="ExternalInput")
    b1c_d = nc.dram_tensor("b1c", [HID, 1], f32, kind="ExternalInput")
    bcatc_d = nc.dram_tensor("bcatc", [O2, 1], f32, kind="ExternalInput")
    dshc_d = nc.dram_tensor("dshc", [128, NSH // 128], f32, kind="ExternalInput")
    drep_d = nc.dram_tensor("drep", [128, T * 128], f32, kind="ExternalInput")
    downc_d = nc.dram_tensor("downc", [128, T], f32, kind="ExternalInput")
    iota_d = nc.dram_tensor("iota", [128, 128], bf16, kind="ExternalInput")
    ident_d = nc.dram_tensor("ident", [128, 128], bf16, kind="ExternalInput")
    idx_d = nc.dram_tensor("idx", [128, L // 16], mybir.dt.int16, kind="ExternalInput")
    dstloc_d = nc.dram_tensor("dstloc", [128, K_tot], bf16, kind="ExternalInput")

    g1s_d = nc.dram_tensor("g1s", [NSH, HID], bf16)
    g2s_d = nc.dram_tensor("g2s", [NPC, HID], bf16)
    if cfg["ag_shared"]:
        g1f_sh = nc.dram_tensor("g1fsh", [NPAD, HID], bf16, addr_space="Shared")
        g2f_sh = nc.dram_tensor("g2fsh", [N, HID], bf16, addr_space="Shared")
    g1f_d = nc.dram_tensor("g1f", [NPAD, HID], bf16)
    g2f_d = nc.dram_tensor("g2f", [N, HID], bf16)
    outt_d = nc.dram_tensor("outt", [O2, T * 128], f32, kind="ExternalOutput")

    def pair_view(dram_t, n_pairs):
        ap = dram_t[:, :]
        return bass.AP(ap.tensor, 0, [[256, n_pairs], [1, 256]])

    with tile.TileContext(nc, trace_sim=bool(cfg.get("trace_sim"))) as tc:
        _emit(nc, tc, cfg, meta, locals())
    nc.compile()
    return nc


def _emit(nc, tc, cfg, meta, env):
    N, C = cfg["n"], cfg["n_cores"]
    NPC = N // C
    T = _ceil(NPC, 128)
    NPAD = cfg["npad"]
    NSH = NPAD // C
    FIN, HID, O2 = cfg["fin"], cfg["hid"], cfg["out2"]
    BLK, G = cfg["blk"], cfg["g_edges"]
    KC = FIN // 128
    SPC = G // 128
    gpt, L, K_tot = meta["gpt"], meta["L"], meta["K_tot"]
    bf16 = mybir.dt.bfloat16
    f32 = mybir.dt.float32
    AF = mybir.ActivationFunctionType
    OP = mybir.AluOpType
    xt_d, w1t_d, wcatt_d = env["xt_d"], env["w1t_d"], env["wcatt_d"]
    b1c_d, bcatc_d = env["b1c_d"], env["bcatc_d"]
    dshc_d, drep_d, downc_d = env["dshc_d"], env["drep_d"], env["downc_d"]
    iota_d, ident_d, idx_d, dstloc_d = env["iota_d"], env["ident_d"], env["idx_d"], env["dstloc_d"]
    g1s_d, g2s_d, g1f_d, g2f_d, outt_d = (env["g1s_d"], env["g2s_d"], env["g1f_d"],
                                          env["g2f_d"], env["outt_d"])
    g1f_sh = env.get("g1f_sh")
    g2f_sh = env.get("g2f_sh")
    pair_view = env["pair_view"]

    with tc.tile_pool(name="const", bufs=1) as const_p:
            w1t_sb = []
            for kc in range(KC):
                w = const_p.tile([128, HID], bf16, tag=f"w1t{kc}")
                nc.sync.dma_start(w[:], w1t_d[kc * 128:(kc + 1) * 128, :])
                w1t_sb.append(w)
            wcatt_sb = const_p.tile([HID, O2], bf16, tag="wcatt")
            nc.sync.dma_start(wcatt_sb[:], wcatt_d[:])
            b1_sb = const_p.tile([HID, 1], f32, tag="b1")
            nc.sync.dma_start(b1_sb[:], b1c_d[:])
            bcat_sb = const_p.tile([O2, 1], f32, tag="bcat")
            nc.sync.dma_start(bcat_sb[:], bcatc_d[:])
            dshc_sb = const_p.tile([128, NSH // 128], f32, tag="dshc")
            nc.sync.dma_start(dshc_sb[:], dshc_d[:])
            downc_sb = const_p.tile([128, T], f32, tag="downc")
            nc.sync.dma_start(downc_sb[:], downc_d[:])
            iota_sb = const_p.tile([128, 128], bf16, tag="iota")
            nc.sync.dma_start(iota_sb[:], iota_d[:])
            ident_sb = const_p.tile([128, 128], bf16, tag="ident")
            nc.sync.dma_start(ident_sb[:], ident_d[:])
            # large non-phase-A constants: DMAs issued after the x-shard load below
            drep_sb = const_p.tile([128, T * 128], f32, tag="drep")
            idx_sb = const_p.tile([128, L // 16], mybir.dt.int16, tag="idx")
            dstloc_sb = const_p.tile([128, K_tot], bf16, tag="dstloc")

            # ---------------- phase A: g1 shard = d * (x_shard @ W1.T)
            scA, _ = nc.enter_named_scope("phaseA", False)
            with tc.tile_pool(name="pa_x", bufs=1) as xt_p, \
                 tc.tile_pool(name="pa_t", bufs=3) as t1_p, \
                 tc.tile_pool(name="pa_w", bufs=3) as wst_p, \
                 tc.tile_pool(name="pa_ps", bufs=2, space="PSUM") as pa, \
                 tc.tile_pool(name="pa_ps2", bufs=2, space="PSUM") as pb:
                xall = xt_p.tile([128, KC, NSH], bf16, tag="xall")
                nc.sync.dma_start(
                    xall[:], xt_d[:, :].rearrange("(kc p) w -> p kc w", p=128))
                nc.sync.dma_start(drep_sb[:], drep_d[:])
                nc.sync.dma_start(idx_sb[:], idx_d[:])
                nc.sync.dma_start(dstloc_sb[:], dstloc_d[:])
                for blki in range(NSH // BLK):
                    ps_a = pa.tile([128, BLK], f32, space="PSUM", tag="psa")
                    for kc in range(KC):
                        nc.tensor.matmul(
                            ps_a[:], lhsT=w1t_sb[kc][:],
                            rhs=xall[:, kc, blki * BLK:(blki + 1) * BLK],
                            start=(kc == 0), stop=(kc == KC - 1))
                    t1t = t1_p.tile([128, BLK], bf16, tag="t1t")
                    nc.scalar.copy(t1t[:], ps_a[:])
                    sb = BLK // 128
                    ps_b = pb.tile([128, sb, 128], bf16, space="PSUM", tag="psb")
                    for s in range(sb):
                        nc.tensor.transpose(ps_b[:, s, :], t1t[:, s * 128:(s + 1) * 128],
                                            ident_sb[:])
                    wst = wst_p.tile([128, sb, HID], bf16, tag="wst")
                    # wst[p, s, f] = ps_b[p, s, f] * d[blk0 + s*128 + p]
                    nb0 = blki * sb
                    dsl = dshc_sb[:, nb0:nb0 + sb]
                    in1 = bass.AP(dsl.tensor, dsl.offset,
                                  [dsl.ap[0], [dsl.ap[1][0], sb], [0, 128]])
                    nc.vector.tensor_tensor(out=wst[:], in0=ps_b[:], in1=in1, op=OP.mult)
                    r0 = blki * BLK
                    nc.sync.dma_start(
                        g1s_d[r0:r0 + BLK, :].rearrange("(s p) f -> p s f", p=128), wst[:])
            # AllGather the shard table
            if cfg["ag_shared"]:
                nc.gpsimd.collective_compute(
                    "AllGather", OP.bypass, replica_groups=[list(range(C))],
                    ins=[g1s_d[:]], outs=[g1f_sh[:]])
                nc.sync.dma_start(g1f_d[:, :], g1f_sh[:, :])
            else:
                nc.gpsimd.collective_compute(
                    "AllGather", OP.bypass, replica_groups=[list(range(C))],
                    ins=[g1s_d[:]], outs=[g1f_d[:]])
            nc.leave_named_scope("phaseA", scA, False)
            if cfg.get("stop_after") == "A":
                _drain_out(nc, tc, outt_d)
                return

            # gather size schedule: ramp-up then steady-state G, small remainder last
            sizes = []
            rem = L
            for r in cfg["g_ramp"]:
                if rem <= r:
                    break
                sizes.append(r)
                rem -= r
            nfull, tail_sz = divmod(rem, G)
            sizes += [G] * nfull
            if tail_sz:
                sizes.append(tail_sz)
            starts = np.concatenate([[0], np.cumsum(sizes)])[:-1]
            # chunk -> (gather id, slot within gather)
            chunk_map = []
            for gi, (s0, sz) in enumerate(zip(starts, sizes)):
                for sl in range(sz // 128):
                    chunk_map.append((gi, sl))

            # ---------------- message passing (both layers)
            def propagate(table_pairs, finalize):
                qrr = cfg["queue_rr"]
                with tc.tile_pool(name="mp_g", bufs=cfg["gather_bufs"]) as gath_p, \
                     tc.tile_pool(name="mp_gs", bufs=1) as gaths_p, \
                     tc.tile_pool(name="mp_oh", bufs=3) as oh_p, \
                     tc.tile_pool(name="mp_ps", bufs=4, space="PSUM") as psp:
                    gh = []
                    for gi, (s0, sz) in enumerate(zip(starts, sizes)):
                        if sz == G:
                            gt = gath_p.tile([128, sz // 128, 256], bf16,
                                             tag="gt", name="gt")
                        else:
                            gt = gaths_p.tile([128, sz // 128, 256], bf16,
                                              tag=f"gtr{gi}", name="gt")
                        nc.gpsimd.dma_gather(
                            out_ap=gt[:],
                            in_ap=table_pairs,
                            idxs_ap=idx_sb[:, s0 // 16:s0 // 16 + sz // 16],
                            num_idxs=sz,
                            num_idxs_reg=sz,
                            elem_size=256,
                            single_packet=(sz <= 1024),
                            queue_num=qrr[gi % len(qrr)],
                        )
                        gh.append(gt)
                    kk = 0  # global chunk counter
                    kg = 0  # dstloc column counter
                    for t in range(T):
                        nch_t = gpt[t][0] + gpt[t][1]
                        ps_t = psp.tile([128, 128], f32, space="PSUM", tag="ps", name="ps_t")
                        j = 0
                        for h in (0, 1):
                            nch = gpt[t][h]
                            if nch == 0:
                                continue
                            oh = oh_p.tile([128, nch, 128], bf16, tag="oh", name="oh")
                            dsl = dstloc_sb[:, kg:kg + nch]
                            in0 = bass.AP(dsl.tensor, dsl.offset,
                                          [dsl.ap[0], [dsl.ap[1][0], nch], [0, 128]])
                            io = iota_sb[:]
                            in1 = bass.AP(io.tensor, io.offset,
                                          [io.ap[0], [0, nch], io.ap[1]])
                            nc.vector.tensor_tensor(out=oh[:], in0=in0, in1=in1,
                                                    op=OP.is_equal)
                            for jj in range(nch):
                                gi, sl = chunk_map[kk]
                                gt = gh[gi]
                                nc.tensor.matmul(
                                    ps_t[:],
                                    lhsT=gt[:, sl, h * 128:(h + 1) * 128],
                                    rhs=oh[:, jj, :],
                                    start=(j == 0), stop=(j == nch_t - 1))
                                kk += 1
                                j += 1
                            kg += nch
                        if nch_t == 0:
                            nc.vector.memset(ps_t[:], 0.0)
                        finalize(t, ps_t)

            with tc.tile_pool(name="fin", bufs=4) as fin_p:
                # prop1 finalize: h tile + inline phase C (g2 row block into g2all)
                with tc.tile_pool(name="ht", bufs=2) as ht_p, \
                     tc.tile_pool(name="g2a", bufs=1) as g2a_p, \
                     tc.tile_pool(name="pc_ps", bufs=2, space="PSUM") as pc1, \
                     tc.tile_pool(name="pc_ps2", bufs=2, space="PSUM") as pc2:
                    g2all = g2a_p.tile([128, T, O2], bf16, tag="g2all")

                    def fin1(t, acc_t):
                        tmp = fin_p.tile([128, 128], f32, tag="tmp")
                        nc.vector.tensor_tensor(out=tmp[:], in0=acc_t[:],
                                                in1=drep_sb[:, t * 128:(t + 1) * 128],
                                                op=OP.mult)
                        h_t = ht_p.tile([128, 128], bf16, tag="ht")
                        nc.scalar.activation(h_t[:], tmp[:], AF.Relu, bias=b1_sb[:])
                        ps = pc1.tile([O2, 128], f32, space="PSUM", tag="c1")
                        nc.tensor.matmul(ps[:], lhsT=wcatt_sb[:], rhs=h_t[:],
                                         start=True, stop=True)
                        c_sb = fin_p.tile([O2, 128], bf16, tag="csb")
                        nc.scalar.copy(c_sb[:], ps[:])
                        ps2 = pc2.tile([128, O2], bf16, space="PSUM", tag="c2")
                        nc.tensor.transpose(ps2[:], c_sb[:], ident_sb[:])
                        nc.vector.tensor_scalar_mul(g2all[:, t, :], ps2[:],
                                                    downc_sb[:, t:t + 1])

                    scP1, _ = nc.enter_named_scope("prop1", False)
                    propagate(pair_view(g1f_d, NPAD // 2), fin1)
                    nc.leave_named_scope("prop1", scP1, False)
                    if cfg.get("stop_after") == "P1":
                        _drain_out(nc, tc, outt_d)
                        return

                    # ---------------- phase C: write shard, AllGather
                    scC, _ = nc.enter_named_scope("phaseC", False)
                    lt = NPC // 128  # full tiles; rows lt*128..NPC are the partial tail
                    nc.sync.dma_start(
                        g2s_d[:lt * 128, :].rearrange("(t p) f -> p t f", p=128),
                        g2all[:, :lt, :])
                    if NPC % 128:
                        nc.sync.dma_start(g2s_d[lt * 128:NPC, :],
                                          g2all[:NPC - lt * 128, lt, :])
                    if cfg["ag_shared"]:
                        nc.gpsimd.collective_compute(
                            "AllGather", OP.bypass, replica_groups=[list(range(C))],
                            ins=[g2s_d[:]], outs=[g2f_sh[:]])
                        nc.sync.dma_start(g2f_d[:, :], g2f_sh[:, :])
                    else:
                        nc.gpsimd.collective_compute(
                            "AllGather", OP.bypass, replica_groups=[list(range(C))],
                            ins=[g2s_d[:]], outs=[g2f_d[:]])
                    nc.leave_named_scope("phaseC", scC, False)

                # ---------------- phase D: second propagate + output
                with tc.tile_pool(name="oall", bufs=1) as oall_p:
                    outall = oall_p.tile([O2, T, 128], f32, tag="outall")

                    def fin2(t, acc_t):
                        tmp = fin_p.tile([128, 128], f32, tag="tmp")
                        nc.vector.tensor_tensor(out=tmp[:], in0=acc_t[:],
                                                in1=drep_sb[:, t * 128:(t + 1) * 128],
                                                op=OP.mult)
                        nc.scalar.activation(outall[:, t, :], tmp[:], AF.Identity,
                                             bias=bcat_sb[:])

                    scP2, _ = nc.enter_named_scope("prop2", False)
                    propagate(pair_view(g2f_d, N // 2), fin2)
                    nc.sync.dma_start(outt_d[:, :].rearrange("p (t c) -> p t c", t=T),
                                      outall[:])
                    nc.leave_named_scope("prop2", scP2, False)


def _drain_out(nc, tc, outt_d):
    """Make truncated (stop_after) programs still produce the output tensor."""
    with tc.tile_pool(name="drain", bufs=1) as dp:
        z = dp.tile([128, 16], mybir.dt.float32, tag="z")
        nc.vector.memset(z[:], 0.0)
        nc.sync.dma_start(outt_d[:, 0:16], z[:])


def run(cfg, x, edge_index, W1, b1, W_mu, b_mu, W_logstd, b_logstd, program_cache=None,
        trace=False, result_box=None):
    meta, in_maps = preprocess(cfg, x, edge_index, W1, b1, W_mu, b_mu, W_logstd, b_logstd)
    nc = build_program(cfg, meta)
    res = run_bass_kernel_spmd(nc, in_maps, list(range(cfg["n_cores"])), trace=trace)
    if result_box is not None:
        result_box.append(res)
    N, C = cfg["n"], cfg["n_cores"]
    NPC = N // C
    O = cfg["out2"] // 2
    mu = np.empty((N, O), np.float32)
    logstd = np.empty((N, O), np.float32)
    for c in range(C):
        ot = res.results[c]["outt"]
        mu[c * NPC:(c + 1) * NPC] = ot[:O, :NPC].T
        logstd[c * NPC:(c + 1) * NPC] = ot[O:, :NPC].T
    return mu, logstd


def kernel(x, edge_index, W1, b1, W_mu, b_mu, W_logstd, b_logstd):
    mu, logstd = run(FULL_CFG, x, edge_index, W1, b1, W_mu, b_mu, W_logstd, b_logstd)
    return mu, logstd


# revision 42
# speedup vs baseline: 1.0510x; 1.0510x over previous
"""Trainium2 Bass kernel for a 2-layer GCN encoder (GCNConv -> ReLU -> {GCNConv mu, GCNConv logstd}).

Strategy (8 NeuronCores, SPMD):
  - Math: propagate(M) = D^-1/2 (A+I) D^-1/2 M  ==  d * ((A+I) @ (d * M)) with d = deg^-1/2,
    so per-edge norm weights disappear: scale rows by d before and after message passing.
  - Layers 2 and 3 share the propagate: fuse W_mu/W_logstd into one [128,128] matmul + one
    message-passing pass over 128 features, split on the host afterwards.
  - Sharding: nodes are partitioned across the 8 cores (dst-sharding). Each core owns
    N/8 = 6250 output rows and processes the ~E/8 edges pointing into them.
  - Layer-1 linear (x @ W1.T) is sharded: each core computes NPAD/8 table rows, then an
    AllGather (direct to plain DRAM) replicates the table for gathering. Layer-2 likewise.
  - Message passing: dma_gather (HW gather, int16 indices) pulls PAIR rows (2 nodes, 512B)
    from the HBM table viewed as [NPAD/2, 256]; idx = src>>1 < 32767 so no table split.
    Edges are bucketed by (dst tile, src parity) so each 128-edge chunk uses one parity
    half of the gathered rows as matmul lhsT. A one-hot selection matrix (DVE is_equal vs
    iota) turns the segment-sum into PE matmuls accumulated in PSUM per 128-dst-node tile.
  - Gathers are issued round-robin on 4 SWDGE queues: descriptor generation for different
    queues runs concurrently on different Q7 core groups (~3x faster than one queue).
  - All cores run the same program (SPMD): per-(tile,parity) group sizes padded to the
    max over cores.

kernel(**inputs) takes the full-size inputs and returns (mu, logstd) as float32 numpy arrays.
"""
import sys

sys.path.insert(0, "/opt/trn_rl_repo")

import numpy as np
import ml_dtypes

import concourse.bass as bass
import concourse.bacc as bacc
import concourse.mybir as mybir
import concourse.tile as tile
from concourse.bass_utils import run_bass_kernel_spmd

BF16 = ml_dtypes.bfloat16

# ---------------- configuration ----------------
FULL_CFG = dict(
    n=50000,        # nodes
    fin=512,        # input features
    hid=128,        # hidden features
    out2=128,       # fused mu+logstd features
    n_cores=8,
    npad=53248,     # padded node count (multiple of 512*8; NSH=6656=13*512 per core)
    hsplit=25000,   # node-table half split (both halves' gather idxs < 32768)
    blk=512,        # phase-A block width
    g_edges=4096,   # steady-state gather size (edges per dma_gather)
    g_ramp=(1024, 1024, 1024, 2048),      # per-half startup ramp sizes
    g_tail=(2048, 1024),                  # per-half wind-down sizes
    swdge_queues=4,
    gather_bufs=5,  # bufs per half for the steady-state gather size
    queue_rr=(0, 1, 2, 3),  # round-robin queue assignment for gathers
    ag_shared=False,        # AllGather to Shared scratchpad + bounce (fallback)
)


def _ceil(a, b):
    return -(-a // b)


def preprocess(cfg, x, edge_index, W1, b1, W_mu, b_mu, W_logstd, b_logstd):
    """Host-side: degrees, edge bucketing/padding, operand staging. Returns
    (meta, in_maps). Pure index/layout work plus parameter reformatting."""
    N, C = cfg["n"], cfg["n_cores"]
    NPC = N // C
    T = _ceil(NPC, 128)
    NPAD = cfg["npad"]
    NSH = NPAD // C

    x = np.asarray(x, np.float32)
    ei = np.asarray(edge_index).astype(np.int64)
    W1 = np.asarray(W1, np.float32)
    b1 = np.asarray(b1, np.float32)
    Wcat = np.concatenate([np.asarray(W_mu, np.float32), np.asarray(W_logstd, np.float32)], axis=0)
    bcat = np.concatenate([np.asarray(b_mu, np.float32), np.asarray(b_logstd, np.float32)], axis=0)

    HS = cfg["hsplit"]
    src = np.concatenate([ei[0], np.arange(N, dtype=np.int64)])
    dst = np.concatenate([ei[1], np.arange(N, dtype=np.int64)])
    deg = np.bincount(dst, minlength=N).astype(np.float32)
    dvec = (1.0 / np.sqrt(deg)).astype(np.float32)

    core = dst // NPC
    tloc = (dst % NPC) // 128
    half = (src >= HS).astype(np.int64)
    key = (core * T + tloc) * 2 + half
    order = np.lexsort((src, key))  # group by (core,tile,half), ascending src inside
    ss, ds = src[order], dst[order]
    counts = np.bincount(key, minlength=C * T * 2).reshape(C, T, 2)
    gpad = ((counts.max(axis=0) + 127) // 128) * 128  # [T, 2] padded group sizes
    Lh = gpad.sum(axis=0).astype(int)  # per-half padded totals
    L = int(Lh.sum())
    K_tot = L // 128
    offs = np.concatenate([[0], np.cumsum(counts.reshape(-1))])

    # phase-A staging (per-core x slice)
    xt_full = np.zeros((cfg["fin"], NPAD), BF16)
    xt_full[:, :N] = x.T
    w1t = np.ascontiguousarray(W1.T).astype(BF16)          # [fin, hid]
    wcatt = np.ascontiguousarray(Wcat.T).astype(BF16)      # [hid, out2]
    d_all = np.ones(NPAD, np.float32)
    d_all[:N] = dvec
    iota_arr = np.tile(np.arange(128), (128, 1)).astype(BF16)
    ident = np.eye(128, dtype=BF16)

    in_maps = []
    for c in range(C):
        bufs_ = [np.zeros(max(int(Lh[0]), 16), np.int16),
                 np.zeros(max(int(Lh[1]), 16), np.int16)]
        ph = [0, 0]
        dstloc_all = np.full(L, 200, np.int32)
        pos = 0
        for t in range(T):
            for h in (0, 1):
                g = int(counts[c, t, h])
                o = int(offs[(c * T + t) * 2 + h])
                sl = slice(o, o + g)
                bufs_[h][ph[h]:ph[h] + g] = (ss[sl] - h * HS).astype(np.int16)
                dstloc_all[pos:pos + g] = (ds[sl] % NPC) - t * 128
                ph[h] += int(gpad[t, h])
                pos += int(gpad[t, h])
        idx_w = [np.tile(b.reshape(-1, 16).T, (8, 1)).copy() for b in bufs_]
        dstloc_arr = np.ascontiguousarray(dstloc_all.reshape(-1, 128).T).astype(BF16)

        d_own = dvec[c * NPC:(c + 1) * NPC]
        d_own_pad = np.ones(T * 128, np.float32)
        d_own_pad[:NPC] = d_own
        d_rep = np.tile(d_own_pad, (128, 1)).astype(np.float32)          # [128, T*128]
        d_own_col = np.ascontiguousarray(d_own_pad.reshape(-1, 128).T)   # [128, T]

        d_sh = d_all[c * NSH:(c + 1) * NSH]
        d_sh_col = np.ascontiguousarray(d_sh.reshape(-1, 128).T)         # [128, NSH/128]

        in_maps.append({
            "xt": np.ascontiguousarray(xt_full[:, c * NSH:(c + 1) * NSH]),
            "w1t": w1t, "wcatt": wcatt,
            "b1c": b1.reshape(-1, 1).copy(), "bcatc": bcat.reshape(-1, 1).copy(),
            "dshc": d_sh_col, "drep": d_rep, "downc": d_own_col,
            "iota": iota_arr, "ident": ident,
            "idx0": idx_w[0], "idx1": idx_w[1], "dstloc": dstloc_arr,
        })

    gpt = [[int(gpad[t, 0]) // 128, int(gpad[t, 1]) // 128] for t in range(T)]
    meta = dict(gpt=gpt, L=L, Lh=[int(Lh[0]), int(Lh[1])], K_tot=K_tot)
    return meta, in_maps


def build_program(cfg, meta):
    N, C = cfg["n"], cfg["n_cores"]
    NPC = N // C
    T = _ceil(NPC, 128)
    NPAD = cfg["npad"]
    NSH = NPAD // C
    FIN, HID, O2 = cfg["fin"], cfg["hid"], cfg["out2"]
    BLK, G = cfg["blk"], cfg["g_edges"]
    KC = FIN // 128
    SPC = G // 128
    gpt, L, Lh, K_tot = meta["gpt"], meta["L"], meta["Lh"], meta["K_tot"]
    bf16 = mybir.dt.bfloat16
    f32 = mybir.dt.float32
    AF = mybir.ActivationFunctionType
    OP = mybir.AluOpType

    nc = bacc.Bacc("TRN2", target_bir_lowering=False, debug=False, num_devices=C,
                   num_swdge_queues=cfg["swdge_queues"])

    xt_d = nc.dram_tensor("xt", [FIN, NSH], bf16, kind="ExternalInput")
    w1t_d = nc.dram_tensor("w1t", [FIN, HID], bf16, kind="ExternalInput")
    wcatt_d = nc.dram_tensor("wcatt", [HID, O2], bf16, kind="ExternalInput")
    b1c_d = nc.dram_tensor("b1c", [HID, 1], f32, kind="ExternalInput")
    bcatc_d = nc.dram_tensor("bcatc", [O2, 1], f32, kind="ExternalInput")
    dshc_d = nc.dram_tensor("dshc", [128, NSH // 128], f32, kind="ExternalInput")
    drep_d = nc.dram_tensor("drep", [128, T * 128], f32, kind="ExternalInput")
    downc_d = nc.dram_tensor("downc", [128, T], f32, kind="ExternalInput")
    iota_d = nc.dram_tensor("iota", [128, 128], bf16, kind="ExternalInput")
    ident_d = nc.dram_tensor("ident", [128, 128], bf16, kind="ExternalInput")
    idx_d = [nc.dram_tensor(f"idx{h}", [128, max(Lh[h] // 16, 1)], mybir.dt.int16,
                            kind="ExternalInput") for h in (0, 1)]
    dstloc_d = nc.dram_tensor("dstloc", [128, K_tot], bf16, kind="ExternalInput")

    g1s_d = nc.dram_tensor("g1s", [NSH, HID], bf16)
    g2s_d = nc.dram_tensor("g2s", [NPC, HID], bf16)
    if cfg["ag_shared"]:
        g1f_sh = nc.dram_tensor("g1fsh", [NPAD, HID], bf16, addr_space="Shared")
        g2f_sh = nc.dram_tensor("g2fsh", [N, HID], bf16, addr_space="Shared")
    g1f_d = nc.dram_tensor("g1f", [NPAD, HID], bf16)
    g2f_d = nc.dram_tensor("g2f", [N, HID], bf16)
    outt_d = nc.dram_tensor("outt", [O2, T * 128], f32, kind="ExternalOutput")

    def half_view(dram_t, row0, nrows):
        ap = dram_t[:, :]
        return bass.AP(ap.tensor, row0 * 128, [[128, nrows], [1, 128]])

    with tile.TileContext(nc, trace_sim=bool(cfg.get("trace_sim"))) as tc:
        _emit(nc, tc, cfg, meta, locals())
    nc.compile()
    return nc


def _emit(nc, tc, cfg, meta, env):
    N, C = cfg["n"], cfg["n_cores"]
    NPC = N // C
    T = _ceil(NPC, 128)
    NPAD = cfg["npad"]
    NSH = NPAD // C
    FIN, HID, O2 = cfg["fin"], cfg["hid"], cfg["out2"]
    BLK, G = cfg["blk"], cfg["g_edges"]
    KC = FIN // 128
    SPC = G // 128
    gpt, L, Lh, K_tot = meta["gpt"], meta["L"], meta["Lh"], meta["K_tot"]
    HS = cfg["hsplit"]
    bf16 = mybir.dt.bfloat16
    f32 = mybir.dt.float32
    AF = mybir.ActivationFunctionType
    OP = mybir.AluOpType
    xt_d, w1t_d, wcatt_d = env["xt_d"], env["w1t_d"], env["wcatt_d"]
    b1c_d, bcatc_d = env["b1c_d"], env["bcatc_d"]
    dshc_d, drep_d, downc_d = env["dshc_d"], env["drep_d"], env["downc_d"]
    iota_d, ident_d, idx_d, dstloc_d = env["iota_d"], env["ident_d"], env["idx_d"], env["dstloc_d"]
    g1s_d, g2s_d, g1f_d, g2f_d, outt_d = (env["g1s_d"], env["g2s_d"], env["g1f_d"],
                                          env["g2f_d"], env["outt_d"])
    g1f_sh = env.get("g1f_sh")
    g2f_sh = env.get("g2f_sh")
    half_view = env["half_view"]

    with tc.tile_pool(name="const", bufs=1) as const_p:
            w1t_sb = []
            for kc in range(KC):
                w = const_p.tile([128, HID], bf16, tag=f"w1t{kc}")
                nc.sync.dma_start(w[:], w1t_d[kc * 128:(kc + 1) * 128, :])
                w1t_sb.append(w)
            wcatt_sb = const_p.tile([HID, O2], bf16, tag="wcatt")
            nc.sync.dma_start(wcatt_sb[:], wcatt_d[:])
            b1_sb = const_p.tile([HID, 1], f32, tag="b1")
            nc.sync.dma_start(b1_sb[:], b1c_d[:])
            bcat_sb = const_p.tile([O2, 1], f32, tag="bcat")
            nc.sync.dma_start(bcat_sb[:], bcatc_d[:])
            dshc_sb = const_p.tile([128, NSH // 128], f32, tag="dshc")
            nc.sync.dma_start(dshc_sb[:], dshc_d[:])
            downc_sb = const_p.tile([128, T], f32, tag="downc")
            nc.sync.dma_start(downc_sb[:], downc_d[:])
            iota_sb = const_p.tile([128, 128], bf16, tag="iota")
            nc.sync.dma_start(iota_sb[:], iota_d[:])
            ident_sb = const_p.tile([128, 128], bf16, tag="ident")
            nc.sync.dma_start(ident_sb[:], ident_d[:])
            # large non-phase-A constants: DMAs issued after the x-shard load below
            drep_sb = const_p.tile([128, T * 128], f32, tag="drep")
            idx_sb = [const_p.tile([128, max(Lh[h] // 16, 1)], mybir.dt.int16,
                                   tag=f"idx{h}", name=f"idx{h}") for h in (0, 1)]
            dstloc_sb = const_p.tile([128, K_tot], bf16, tag="dstloc")

            # ---------------- phase A: g1 shard = d * (x_shard @ W1.T)
            scA, _ = nc.enter_named_scope("phaseA", False)
            with tc.tile_pool(name="pa_x", bufs=1) as xt_p, \
                 tc.tile_pool(name="pa_t", bufs=3) as t1_p, \
                 tc.tile_pool(name="pa_w", bufs=3) as wst_p, \
                 tc.tile_pool(name="pa_ps", bufs=2, space="PSUM") as pa, \
                 tc.tile_pool(name="pa_ps2", bufs=2, space="PSUM") as pb:
                xall = xt_p.tile([128, KC, NSH], bf16, tag="xall")
                nc.sync.dma_start(
                    xall[:], xt_d[:, :].rearrange("(kc p) w -> p kc w", p=128))
                nc.sync.dma_start(drep_sb[:], drep_d[:])
                nc.sync.dma_start(idx_sb[0][:], idx_d[0][:])
                nc.sync.dma_start(idx_sb[1][:], idx_d[1][:])
                nc.sync.dma_start(dstloc_sb[:], dstloc_d[:])
                for blki in range(NSH // BLK):
                    ps_a = pa.tile([128, BLK], f32, space="PSUM", tag="psa")
                    for kc in range(KC):
                        nc.tensor.matmul(
                            ps_a[:], lhsT=w1t_sb[kc][:],
                            rhs=xall[:, kc, blki * BLK:(blki + 1) * BLK],
                            start=(kc == 0), stop=(kc == KC - 1))
                    t1t = t1_p.tile([128, BLK], bf16, tag="t1t")
                    nc.scalar.copy(t1t[:], ps_a[:])
                    sb = BLK // 128
                    ps_b = pb.tile([128, sb, 128], bf16, space="PSUM", tag="psb")
                    for s in range(sb):
                        nc.tensor.transpose(ps_b[:, s, :], t1t[:, s * 128:(s + 1) * 128],
                                            ident_sb[:])
                    wst = wst_p.tile([128, sb, HID], bf16, tag="wst")
                    # wst[p, s, f] = ps_b[p, s, f] * d[blk0 + s*128 + p]
                    nb0 = blki * sb
                    dsl = dshc_sb[:, nb0:nb0 + sb]
                    in1 = bass.AP(dsl.tensor, dsl.offset,
                                  [dsl.ap[0], [dsl.ap[1][0], sb], [0, 128]])
                    nc.vector.tensor_tensor(out=wst[:], in0=ps_b[:], in1=in1, op=OP.mult)
                    r0 = blki * BLK
                    nc.sync.dma_start(
                        g1s_d[r0:r0 + BLK, :].rearrange("(s p) f -> p s f", p=128), wst[:])
            # AllGather the shard table
            if cfg["ag_shared"]:
                nc.gpsimd.collective_compute(
                    "AllGather", OP.bypass, replica_groups=[list(range(C))],
                    ins=[g1s_d[:]], outs=[g1f_sh[:]])
                nc.sync.dma_start(g1f_d[:, :], g1f_sh[:, :])
            else:
                nc.gpsimd.collective_compute(
                    "AllGather", OP.bypass, replica_groups=[list(range(C))],
                    ins=[g1s_d[:]], outs=[g1f_d[:]])
            nc.leave_named_scope("phaseA", scA, False)
            if cfg.get("stop_after") == "A":
                _drain_out(nc, tc, outt_d)
                return

            # per-half gather size schedules: ramp-up, steady-state G, wind-down
            def build_sizes(total):
                up, rem = [], total
                for r in cfg["g_ramp"]:
                    if rem <= r:
                        break
                    up.append(r)
                    rem -= r
                down = []
                for r in cfg["g_tail"]:
                    if rem <= r:
                        break
                    down.append(r)
                    rem -= r
                nfull, tail_sz = divmod(rem, G)
                mid = [G] * nfull + ([tail_sz] if tail_sz else [])
                return up + mid + down

            sizes_h = [build_sizes(Lh[0]), build_sizes(Lh[1])]
            starts_h = [np.concatenate([[0], np.cumsum(s)])[:-1] for s in sizes_h]
            # per half: chunk -> (gather id, slot within gather)
            chunk_map_h = []
            for h in (0, 1):
                cm = []
                for gi, sz in enumerate(sizes_h[h]):
                    cm += [(gi, sl) for sl in range(sz // 128)]
                chunk_map_h.append(cm)
            # interleaved issue order (h, gi) alternating between halves
            issue = []
            mx = max(len(sizes_h[0]), len(sizes_h[1]))
            for gi in range(mx):
                for h in (0, 1):
                    if gi < len(sizes_h[h]):
                        issue.append((h, gi))

            # ---------------- message passing (both layers)
            def propagate(tables, finalize):
                qrr = cfg["queue_rr"]
                with tc.tile_pool(name="mp_g", bufs=cfg["gather_bufs"]) as gath_p, \
                     tc.tile_pool(name="mp_gs", bufs=1) as gaths_p, \
                     tc.tile_pool(name="mp_oh", bufs=3) as oh_p, \
                     tc.tile_pool(name="mp_ps", bufs=4, space="PSUM") as psp:
                    gh = [[None] * len(sizes_h[0]), [None] * len(sizes_h[1])]
                    for qi, (h, gi) in enumerate(issue):
                        s0, sz = int(starts_h[h][gi]), sizes_h[h][gi]
                        if sz == G:
                            gt = gath_p.tile([128, sz // 128, 128], bf16,
                                             tag=f"gt{h}", name="gt")
                        else:
                            gt = gaths_p.tile([128, sz // 128, 128], bf16,
                                              tag=f"gtr{h}_{gi}", name="gt")
                        nc.gpsimd.dma_gather(
                            out_ap=gt[:],
                            in_ap=tables[h],
                            idxs_ap=idx_sb[h][:, s0 // 16:s0 // 16 + sz // 16],
                            num_idxs=sz,
                            num_idxs_reg=sz,
                            elem_size=128,
                            single_packet=(sz <= 1024),
                            queue_num=qrr[qi % len(qrr)],
                        )
                        gh[h][gi] = gt
                    kk = [0, 0]  # per-half chunk counters
                    kg = 0       # dstloc column counter
                    for t in range(T):
                        nch_t = gpt[t][0] + gpt[t][1]
                        ps_t = psp.tile([128, 128], f32, space="PSUM", tag="ps", name="ps_t")
                        j = 0
                        for h in (0, 1):
                            nch = gpt[t][h]
                            if nch == 0:
                                continue
                            oh = oh_p.tile([128, nch, 128], bf16, tag="oh", name="oh")
                            dsl = dstloc_sb[:, kg:kg + nch]
                            in0 = bass.AP(dsl.tensor, dsl.offset,
                                          [dsl.ap[0], [dsl.ap[1][0], nch], [0, 128]])
                            io = iota_sb[:]
                            in1 = bass.AP(io.tensor, io.offset,
                                          [io.ap[0], [0, nch], io.ap[1]])
                            nc.vector.tensor_tensor(out=oh[:], in0=in0, in1=in1,
                                                    op=OP.is_equal)
                            for jj in range(nch):
                                gi, sl = chunk_map_h[h][kk[h]]
                                gt = gh[h][gi]
                                nc.tensor.matmul(
                                    ps_t[:],
                                    lhsT=gt[:, sl, :],
                                    rhs=oh[:, jj, :],
                                    start=(j == 0), stop=(j == nch_t - 1))
                                kk[h] += 1
                                j += 1
                            kg += nch
                        if nch_t == 0:
                            nc.vector.memset(ps_t[:], 0.0)
                        finalize(t, ps_t)

            with tc.tile_pool(name="fin", bufs=4) as fin_p:
                # prop1 finalize: h tile + inline phase C (g2 row block into g2all)
                with tc.tile_pool(name="ht", bufs=2) as ht_p, \
                     tc.tile_pool(name="g2a", bufs=1) as g2a_p, \
                     tc.tile_pool(name="pc_ps", bufs=2, space="PSUM") as pc1, \
                     tc.tile_pool(name="pc_ps2", bufs=2, space="PSUM") as pc2:
                    g2all = g2a_p.tile([128, T, O2], bf16, tag="g2all")

                    def fin1(t, acc_t):
                        tmp = fin_p.tile([128, 128], f32, tag="tmp")
                        nc.vector.tensor_tensor(out=tmp[:], in0=acc_t[:],
                                                in1=drep_sb[:, t * 128:(t + 1) * 128],
                                                op=OP.mult)
                        h_t = ht_p.tile([128, 128], bf16, tag="ht")
                        nc.scalar.activation(h_t[:], tmp[:], AF.Relu, bias=b1_sb[:])
                        ps = pc1.tile([O2, 128], f32, space="PSUM", tag="c1")
                        nc.tensor.matmul(ps[:], lhsT=wcatt_sb[:], rhs=h_t[:],
                                         start=True, stop=True)
                        c_sb = fin_p.tile([O2, 128], bf16, tag="csb")
                        nc.scalar.copy(c_sb[:], ps[:])
                        ps2 = pc2.tile([128, O2], bf16, space="PSUM", tag="c2")
                        nc.tensor.transpose(ps2[:], c_sb[:], ident_sb[:])
                        nc.vector.tensor_scalar_mul(g2all[:, t, :], ps2[:],
                                                    downc_sb[:, t:t + 1])

                    scP1, _ = nc.enter_named_scope("prop1", False)
                    propagate([half_view(g1f_d, 0, HS), half_view(g1f_d, HS, NPAD - HS)],
                              fin1)
                    nc.leave_named_scope("prop1", scP1, False)
                    if cfg.get("stop_after") == "P1":
                        _drain_out(nc, tc, outt_d)
                        return

                    # ---------------- phase C: write shard, AllGather
                    scC, _ = nc.enter_named_scope("phaseC", False)
                    lt = NPC // 128  # full tiles; rows lt*128..NPC are the partial tail
                    nc.sync.dma_start(
                        g2s_d[:lt * 128, :].rearrange("(t p) f -> p t f", p=128),
                        g2all[:, :lt, :])
                    if NPC % 128:
                        nc.sync.dma_start(g2s_d[lt * 128:NPC, :],
                                          g2all[:NPC - lt * 128, lt, :])
                    if cfg["ag_shared"]:
                        nc.gpsimd.collective_compute(
                            "AllGather", OP.bypass, replica_groups=[list(range(C))],
                            ins=[g2s_d[:]], outs=[g2f_sh[:]])
                        nc.sync.dma_start(g2f_d[:, :], g2f_sh[:, :])
                    else:
                        nc.gpsimd.collective_compute(
                            "AllGather", OP.bypass, replica_groups=[list(range(C))],
                            ins=[g2s_d[:]], outs=[g2f_d[:]])
                    nc.leave_named_scope("phaseC", scC, False)

                # ---------------- phase D: second propagate + output
                with tc.tile_pool(name="oall", bufs=1) as oall_p:
                    outall = oall_p.tile([O2, T, 128], f32, tag="outall")

                    def fin2(t, acc_t):
                        tmp = fin_p.tile([128, 128], f32, tag="tmp")
                        nc.vector.tensor_tensor(out=tmp[:], in0=acc_t[:],
                                                in1=drep_sb[:, t * 128:(t + 1) * 128],
                                                op=OP.mult)
                        nc.scalar.activation(outall[:, t, :], tmp[:], AF.Identity,
                                             bias=bcat_sb[:])

                    scP2, _ = nc.enter_named_scope("prop2", False)
                    propagate([half_view(g2f_d, 0, HS), half_view(g2f_d, HS, N - HS)],
                              fin2)
                    nc.sync.dma_start(outt_d[:, :].rearrange("p (t c) -> p t c", t=T),
                                      outall[:])
                    nc.leave_named_scope("prop2", scP2, False)


def _drain_out(nc, tc, outt_d):
    """Make truncated (stop_after) programs still produce the output tensor."""
    with tc.tile_pool(name="drain", bufs=1) as dp:
        z = dp.tile([128, 16], mybir.dt.float32, tag="z")
        nc.vector.memset(z[:], 0.0)
        nc.sync.dma_start(outt_d[:, 0:16], z[:])


def run(cfg, x, edge_index, W1, b1, W_mu, b_mu, W_logstd, b_logstd, program_cache=None,
        trace=False, result_box=None):
    meta, in_maps = preprocess(cfg, x, edge_index, W1, b1, W_mu, b_mu, W_logstd, b_logstd)
    nc = build_program(cfg, meta)
    res = run_bass_kernel_spmd(nc, in_maps, list(range(cfg["n_cores"])), trace=trace)
    if result_box is not None:
        result_box.append(res)
    N, C = cfg["n"], cfg["n_cores"]
    NPC = N // C
    O = cfg["out2"] // 2
    mu = np.empty((N, O), np.float32)
    logstd = np.empty((N, O), np.float32)
    for c in range(C):
        ot = res.results[c]["outt"]
        mu[c * NPC:(c + 1) * NPC] = ot[:O, :NPC].T
        logstd[c * NPC:(c + 1) * NPC] = ot[O:, :NPC].T
    return mu, logstd


def kernel(x, edge_index, W1, b1, W_mu, b_mu, W_logstd, b_logstd):
    mu, logstd = run(FULL_CFG, x, edge_index, W1, b1, W_mu, b_mu, W_logstd, b_logstd)
    return mu, logstd
